# revision 15
# baseline (speedup 1.0000x reference)
"""Trainium2 Bass kernel for DotProductAttention + concat-FC (B=16,Q=1024,S=2048,D=1024).

Strategy
--------
Data-parallel over batch: 16 batches / 8 cores = 2 per core, zero collectives.

Per batch, everything is computed in a TRANSPOSED layout so that no on-device
transposes are needed (all operand layouts are produced host-side):

  m1:  scoresT[s,q] = sum_d V[s,d]*Q[q,d]      lhsT = vT tile [d,s], rhs = qT [d,q]
  softmax over s (= partitions), exploiting shift invariance: exp(x - C) with a
      constant C=128 straight off PSUM on ScalarE, per-(s-partition) partial
      sums chained on VectorE, then one gpsimd partition_all_reduce(add),
      then reciprocal.
  align: the NT8 first s-tiles are normalized (exp*recip) and quantized to
      fp8-e4m3 on VectorE; the remaining s-tiles normalized to bf16. A second
      colsum chain over the *quantized* align values + all_reduce gives recip2,
      which renormalizes away the common-mode fp8 quantization error of align.
  m2:  ctxT[d,q]  = sum_s V[s,d]*align[s,q]
      fp8 s-tile PAIRS via DoubleRow matmuls (2 contraction tiles per MM at
      ~0.5 cyc/row) with e4m3 V; bf16 s-tiles as normal matmuls; one PSUM
      accumulation group per (j,h); drain multiplies by recip2 -> ctxT f16.
      m2 runs h-outer (all h0 j-chunks then h1) so VectorE can produce h1's
      align tiles under m2-h0's PE work; h0's align production interleaves
      into m1-h1's t-loop (it only needs recip1-h0).
  m3:  outT[o,q] = tanh(sum_e fc_w[o,e]*combT[e,q] + b[o])
      combT = [ctxT ; qT] picked per contraction chunk, bias+tanh fused in one
      ScalarE activation on the PSUM drain. fp16 throughout (fp8 here would
      inject ~4% straight into the output; budget is 2e-2).

Numerics: NT8=12 of 16 s-tiles in fp8 gives rel_err 1.748e-2 (numpy-emulated
and HW-verified to 4 digits); full fp8 m2 would be 2.01e-2 — over the 2e-2
budget. m1 must stay fp16: even 1/8 of the score contraction in fp8 flips
argmaxes of the (very peaked, scores~N(0,32)) softmax.

Softmax-stats critical path (the first fp8 attempt lost 22us/batch to it):
  * partition reductions (colsum broadcast) are ONES-MATMULS on the PE
    (ones.T @ x, 213ns) instead of gpsimd partition_all_reduce (4.5us);
  * recip1 is reciprocal_approx_fast (~5x cheaper than reciprocal; ANY
    recip1/colsum error is per-q common-mode on align and cancels exactly
    through recip2's renorm, which is why colsum accumulates in bf16);
  * colsum2 (sum of quantized align) is computed exactly in PSUM by
    ones-matmuls (fp8-DR over al8 pairs + bf16 over alb);
  * DVE queue order at the m1-end boundary: recip1-h1 + gamma cast,
    recip2-h0, drain(j0,h0) (frees its PSUM bank early), then h1's 16
    align mults — m2-h1 is ready ~12us post-m1, inside m2-h0's ~19us
    of PE cover; h0's align mults interleave into m1-h1's per-chain gaps.

Perf notes (fp16 baseline measured ~352us at the fast clock state; fp16 PE
streaming floor for the 1536 N=512 matmuls is ~329us; this version measures
~320.7us: PE busy 306.6us, <1us of PE gaps, ~8us engine-start head, ~5us
sync-epilogue tail. DoubleRow fp8 pairs measure ~248ns vs 2x216ns fp16 —
~1.74x on the converted 12/16 of m2. NOTE the device has a slow clock state
(everything x1.2, e.g. 441us instead of 367us) that persists per-session;
compare runs via the matmul duration histogram: 379ns median = fast state):
  * 16-bit operands for m1/m3 (fp16 q/v/fc_w/ctx, bf16 exp).
  * m1 runs h-outer/t-inner with all 16 V-tiles resident (loaded as t-pairs:
    4KB DMA descriptors), so only qt_h0 (1MB) + one V-pair gates the first
    chain; qt_h1 streams in under the h0 sweep.
  * NWARM dummy N=256 matmuls on memset tiles bridge engine-start (~7.4us) to
    first-data (~13us) so the PE HAM clock-gate (1.2 GHz cold -> 2.4 GHz warm
    after ~3.4us of continuous busy; any ~3us idle re-throttles) is fully
    lifted when real work starts.
  * The DGE rings admit ~3-4 transfers concurrently and FAIR-SHARE bandwidth,
    so every later load is gated behind qt_h0a's arrival via a tiny GpSimd
    write into its tile (WAW dep the scheduler must honor).
  * Loads split across the sync + scalar DGE rings; stores ride the scalar
    ring behind their tanh so they never block load issue.
  * fc_w stays resident in SBUF (4MB fp16) across both batches.
  * m3 contracts the qT half (k=8..15) before the ctxT half so it can start
    before m2's last drains; the final (b1,dt7) group runs its h-chains
    sequentially so the kernel tail is one tanh + one 256KB store.
"""

import sys
import time

if "/opt/trn_rl_repo" not in sys.path:
    sys.path.insert(0, "/opt/trn_rl_repo")

from contextlib import ExitStack

import numpy as np

import concourse.bass as bass  # noqa: F401  (import registers engine classes)
import concourse.mybir as mybir
import concourse.tile as tile
from concourse import bacc
from concourse.bass_utils import run_bass_kernel_spmd

P = 128
B, Q, S, D = 16, 1024, 2048, 1024
NCORES = 8
BL = B // NCORES  # 2 batches per core
QH = Q // 2       # q processed in halves of 512
ST = S // P       # 16 s-tiles
KO = D // P       # 8 contraction chunks over d
KE = 2 * D // P   # 16 contraction chunks over e=2D

NT8 = 14          # s-tiles of m2 computed in fp8-e4m3 (DoubleRow pairs)
NDR = NT8 // 2    # DoubleRow pair count
STB = ST - NT8    # bf16 s-tiles

F32 = mybir.dt.float32
F16 = mybir.dt.float16
BF16 = mybir.dt.bfloat16
F8 = mybir.dt.float8e4
DR = mybir.MatmulPerfMode.DoubleRow

# Constant softmax shift: scores ~ N(0, sqrt(D)=32) so row maxes sit in
# [~70, ~190]; exp(x-128) stays comfortably inside fp32/bf16 range both ways.
SOFTMAX_SHIFT = 128.0

NWARM = 34  # dummy matmuls (N=256) spanning the head DMA window: the HAM
# clock gate needs ~3.4us of continuous PE busy to lift (1.2 -> 2.4 GHz), and
# any >~2.5us idle afterwards re-throttles, so the dummies must bridge all the
# way from engine-start (~7.8us) to first-data (~14.4us)

_COMPILED = None


def _build_kernel(ctx: ExitStack, tc: "tile.TileContext", qT_d, vT_d, vN8_d, vNb_d, fw_d, fb_d, outT_d):
    nc = tc.nc
    consts = ctx.enter_context(tc.tile_pool(name="consts", bufs=1))
    qt_pool = ctx.enter_context(tc.tile_pool(name="qt", bufs=4))
    vt_pool = ctx.enter_context(tc.tile_pool(name="vt", bufs=ST // 2))
    # exps as per-t tiles: at h1-t the window of live tiles is h0[t..15] +
    # h1[0..t] = 17 (h0-t's last reader is its align mult at h1-t), so 18
    # buffers let h1's writes reuse h0's buffers as the sweep progresses.
    pexp = ctx.enter_context(tc.tile_pool(name="pexp", bufs=ST + 2))
    al8_pool = ctx.enter_context(tc.tile_pool(name="al8", bufs=2))
    alb_pool = ctx.enter_context(tc.tile_pool(name="alb", bufs=2))
    stats = ctx.enter_context(tc.tile_pool(name="stats", bufs=2))
    ctx_pool = ctx.enter_context(tc.tile_pool(name="ctxT", bufs=KO))
    vc8_pool = ctx.enter_context(tc.tile_pool(name="vc8", bufs=KO))
    vcb_pool = ctx.enter_context(tc.tile_pool(name="vcb", bufs=KO))
    fw_pool = ctx.enter_context(tc.tile_pool(name="fw", bufs=1))
    outp = ctx.enter_context(tc.tile_pool(name="outp", bufs=2))
    psum = ctx.enter_context(tc.tile_pool(name="psum", bufs=8, space="PSUM"))

    # ---- PE warm-up: dummy matmuls with no DMA dependency ----
    wl = consts.tile([P, P], F16)
    wr = consts.tile([P, QH // 2], F16)
    nc.vector.memset(wl[:], 0.0)
    nc.vector.memset(wr[:], 0.0)
    wp = psum.tile([P, QH], F32, tag="psum", name="warm")[:, : QH // 2]
    for _ in range(NWARM):
        nc.tensor.matmul(wp[:], wl[:], wr[:], start=True, stop=True)

    shift = consts.tile([P, 1], F32)
    nc.vector.memset(shift[:], -float(SOFTMAX_SHIFT))
    fbt = consts.tile([P, KO], F32)
    fwt = fw_pool.tile([P, KO, KE, P], F16)
    # all-ones stationaries: partition reduction as a single PE matmul
    # (ones.T @ x broadcasts sum_p x[p,q] to every output partition) --
    # replaces the 4.5us gpsimd partition_all_reduce on the critical path
    ones_bf = consts.tile([P, P], BF16)
    nc.vector.memset(ones_bf[:], 1.0)
    ones8 = consts.tile([P, 2, P], F8)
    nc.vector.memset(ones8[:], 1.0)

    half = KO // 2
    batches = []

    def emit_loads(b):
        """Batch b's wave-1 loads + (b0) the WAW gates for everything else.

        sync ring:   qt_h0 halves; scalar ring: first two vt pairs + fc_b.
        The rest of the loads are emitted inside emit_m1's t-loop and, for
        b0, WAW-gated on qt_h0a so they can't fair-share against wave-1.
        """
        st = {}
        st["qth"] = [
            qt_pool.tile([P, KO, QH], F16, tag="qt", name=f"qt_{b}_{h}")
            for h in range(2)
        ]
        st["vtp"] = [
            vt_pool.tile([P, 2, KO, P], F16, tag="vt", name=f"vtp_{b}_{tp}")
            for tp in range(ST // 2)
        ]
        st["vcs8"] = [
            vc8_pool.tile([P, NT8, P], F8, tag="vc8", name=f"vc8_{b}_{j}")
            for j in range(KO)
        ]
        st["vcsb"] = [
            vcb_pool.tile([P, STB, P], BF16, tag="vcb", name=f"vcb_{b}_{j}")
            for j in range(KO)
        ]
        # wave-1: ONLY what the first h0 chains need. The DGE rings admit
        # ~3-4 transfers concurrently and fair-share bandwidth between them,
        # so anything issued here delays the critical first bytes 1:1.
        nc.sync.dma_start(st["qth"][0][:, :half, :], qT_d[b, 0, :, :half, :])
        nc.scalar.dma_start(st["vtp"][0][:], vT_d[b, 0])
        nc.sync.dma_start(st["qth"][0][:, half:, :], qT_d[b, 0, :, half:, :])
        nc.scalar.dma_start(st["vtp"][1][:], vT_d[b, 1])
        if b == 0:
            nc.scalar.dma_start(fbt[:], fb_d[:, :])
            # gate every later load behind qt_h0a's arrival: a tiny GpSimd
            # write into each pending tile gives its DMA a WAW dependency, so
            # the DGE rings can't admit them into wave-1, where they would
            # fair-share bandwidth against the critical first megabyte
            gsrc = st["qth"][0][:, 0, :1]
            for tp in range(2, ST // 2):
                nc.gpsimd.tensor_copy(st["vtp"][tp][:, 0, 0, :1], gsrc)
            nc.gpsimd.tensor_copy(st["qth"][1][:, 0, :1], gsrc)
            for j in range(KO):
                nc.gpsimd.tensor_copy(st["vcs8"][j][:, 0, :1], gsrc)
                nc.gpsimd.tensor_copy(st["vcsb"][j][:, 0, :1], gsrc)
            for dt in range(KO):
                nc.gpsimd.tensor_copy(fwt[:, dt, 0, :1], gsrc)
        batches.append(st)
        return st

    def emit_m1(b):
        st = batches[b]
        qth, vtp = st["qth"], st["vtp"]
        exps = st["exps"] = [
            [
                pexp.tile([P, QH], BF16, tag="pexp", name=f"sT_{b}_{h}_{t}")
                for t in range(ST)
            ]
            for h in range(2)
        ]
        al8 = st["al8"] = [
            al8_pool.tile([P, NT8, QH], F8, tag="al8", name=f"al8_{b}_{h}")
            for h in range(2)
        ]
        alb = st["alb"] = [
            alb_pool.tile([P, STB, QH], BF16, tag="alb", name=f"alb_{b}_{h}")
            for h in range(2)
        ]
        # colsum chain accumulates in bf16 (2x DVE; its rounding is a per-q
        # common-mode scale on align that recip2's renorm cancels exactly)
        colsums = st["colsums"] = [
            stats.tile([P, QH], BF16, tag="colsum", name=f"colsum_{b}_{h}")
            for h in range(2)
        ]
        recips1 = st["recips1"] = [
            stats.tile([P, QH], F32, tag="recip1", name=f"recip1_{b}_{h}")
            for h in range(2)
        ]
        gammas = st["gammas"] = [
            stats.tile([P, QH], BF16, tag="gamma", name=f"gamma_{b}_{h}")
            for h in range(2)
        ]
        recips2 = st["recips"] = [
            stats.tile([P, QH], F32, tag="recip2", name=f"recip2_{b}_{h}")
            for h in range(2)
        ]

        def emit_align(h, t):
            # normalized align in the dtype m2 consumes (bf16 x bf16 mult)
            dst = al8[h][:, t, :] if t < NT8 else alb[h][:, t - NT8, :]
            nc.vector.tensor_tensor(
                dst, exps[h][t][:], gammas[h][:], mybir.AluOpType.mult
            )

        st["emit_align"] = emit_align

        for h in range(2):
            for t in range(ST):
                vt = vtp[t // 2][:, t % 2]
                ps = psum.tile([P, QH], F32, tag="psum", name=f"ps_sc_{b}_{h}_{t}")
                for k in range(KO):
                    nc.tensor.matmul(
                        ps[:],
                        vt[:, k, :],
                        qth[h][:, k, :],
                        start=(k == 0),
                        stop=(k == KO - 1),
                    )
                # softmax is shift-invariant: exp(x - C) with a constant C
                nc.scalar.activation(
                    exps[h][t][:],
                    ps[:],
                    mybir.ActivationFunctionType.Exp,
                    bias=shift[:],
                )
                if t == 0:
                    nc.vector.tensor_copy(colsums[h][:], exps[h][0][:])
                else:
                    nc.vector.tensor_tensor(
                        colsums[h][:],
                        colsums[h][:],
                        exps[h][t][:],
                        mybir.AluOpType.add,
                    )
                if h == 1:
                    if t == 0:
                        # h0's softmax stats: ones-matmul allreduce slotted
                        # after h1-t0's chain (covers colsum-h0's DVE tail),
                        # then a cheap approx reciprocal -- any recip1 error
                        # is common-mode per q and cancels via recip2
                        psr = psum.tile([P, QH], F32, tag="psum", name=f"psr_{b}_0")
                        nc.tensor.matmul(
                            psr[:], ones_bf[:], colsums[0][:], start=True, stop=True
                        )
                        nc.vector.reciprocal_approx_fast(recips1[0][:], psr[:])
                        nc.vector.tensor_copy(gammas[0][:], recips1[0][:])
                    else:
                        # h0's align tiles only need gamma-h0: interleave their
                        # DVE ops into h1's sweep (fits in the per-chain gap)
                        emit_align(0, t - 1)
                # remaining loads, gated behind this exp's semaphore wait so
                # their descriptors are admitted at consumption pace and never
                # steal DMA bandwidth from earlier, more urgent transfers
                if h == 0:
                    if t < ST // 2 - 2:
                        nc.scalar.dma_start(vtp[t + 2][:], vT_d[b, t + 2])
                    elif t < ST // 2:
                        qsl = (
                            slice(0, half) if t == ST // 2 - 2 else slice(half, KO)
                        )
                        nc.scalar.dma_start(qth[1][:, qsl, :], qT_d[b, 1, :, qsl, :])
                    else:
                        # vN8 rides the (otherwise idle) sync ring so the
                        # scalar ring keeps room for vt/qt/store traffic
                        nc.sync.dma_start(st["vcs8"][t - ST // 2][:], vN8_d[b, t - ST // 2])
                        nc.scalar.dma_start(st["vcsb"][t - ST // 2][:], vNb_d[b, t - ST // 2])
                elif b == 0 and t < KO:
                    nc.scalar.dma_start(fwt[:, t], fw_d[t])
        emit_align(0, ST - 1)

    def emit_m2(b):
        st = batches[b]
        al8, alb, recips = st["al8"], st["alb"], st["recips"]
        colsums, gammas, recips1 = st["colsums"], st["gammas"], st["recips1"]
        emit_align = st["emit_align"]
        ctxTs = st["ctxTs"] = [
            ctx_pool.tile([P, Q], F16, tag="ctxT", name=f"ctxT_{b}_{j}")
            for j in range(KO)
        ]
        scratch = stats.tile([P, QH], F32, tag="scratch", name=f"scr_{b}")

        def colsum2(h):
            # sum of the QUANTIZED align values, exactly, in PSUM via
            # ones-matmuls: recip2 renormalizes ctx by the true align mass
            psc = psum.tile([P, QH], F32, tag="psum", name=f"psc_{b}_{h}")
            for p in range(NDR):
                nc.tensor.matmul(
                    psc[:],
                    ones8[:],
                    al8[h][:, 2 * p : 2 * p + 2, :],
                    start=(p == 0),
                    stop=False,
                    perf_mode=DR,
                )
            for tb in range(STB):
                nc.tensor.matmul(
                    psc[:],
                    ones_bf[:],
                    alb[h][:, tb, :],
                    start=False,
                    stop=(tb == STB - 1),
                )
            nc.vector.reciprocal_approx_accurate(recips[h][:], psc[:], scratch[:])

        def drain(j, h):
            nc.vector.tensor_tensor(
                ctxTs[j][:, h * QH : (h + 1) * QH],
                psums2[j][h][:],
                recips[h][:],
                mybir.AluOpType.mult,
            )

        psums2 = [[None, None] for _ in range(KO)]
        # h-outer: all of h0's j-chunks run while DVE produces h1's aligns
        for h in range(2):
            for j in range(KO):
                vc8, vcb = st["vcs8"][j], st["vcsb"][j]
                ps = psum.tile([P, QH], F32, tag="psum", name=f"ps_ctx_{b}_{j}_{h}")
                psums2[j][h] = ps
                for p in range(NDR):
                    nc.tensor.matmul(
                        ps[:],
                        vc8[:, 2 * p : 2 * p + 2, :],
                        al8[h][:, 2 * p : 2 * p + 2, :],
                        start=(p == 0),
                        stop=False,
                        perf_mode=DR,
                    )
                for tb in range(STB):
                    nc.tensor.matmul(
                        ps[:],
                        vcb[:, tb, :],
                        alb[h][:, tb, :],
                        start=False,
                        stop=(tb == STB - 1),
                    )
                if h == 0 and j == 0:
                    # PE: h1's allreduce + h0's colsum2, under m2-h0's cover.
                    # DVE order matters (in-order queue): recip1-h1 + gamma,
                    # recip2-h0, drain(0,0) [frees its PSUM bank early], THEN
                    # the 16 h1 align mults -- m2-h1 starts ~12us post-m1,
                    # well inside m2-h0's ~25us of PE work.
                    psr = psum.tile([P, QH], F32, tag="psum", name=f"psr_{b}_1")
                    nc.tensor.matmul(
                        psr[:], ones_bf[:], colsums[1][:], start=True, stop=True
                    )
                    nc.vector.reciprocal_approx_fast(recips1[1][:], psr[:])
                    nc.vector.tensor_copy(gammas[1][:], recips1[1][:])
                    colsum2(0)
                    drain(0, 0)
                    for t in range(ST):
                        emit_align(1, t)
                elif h == 0 and j == 5:
                    colsum2(1)
                    drain(j, h)
                else:
                    drain(j, h)

    def emit_m3(b):
        st = batches[b]
        qth, ctxTs = st["qth"], st["ctxTs"]
        # contract the qT half first: it has no dependency on m2's drains
        korder = list(range(KO, KE)) + list(range(KO))
        for dt in range(KO):
            ps = [
                psum.tile([P, QH], F32, tag="psum", name=f"ps_out_{b}_{dt}_{h}")
                for h in range(2)
            ]
            # the final group (dt=7 of the last batch) runs its h-chains
            # sequentially so the kernel tail is one tanh + one 256KB store
            last = b == BL - 1 and dt == KO - 1
            ihk = (
                [(i, h, k) for h in range(2) for i, k in enumerate(korder)]
                if last
                else [(i, h, k) for i, k in enumerate(korder) for h in range(2)]
            )
            ot = outp.tile([P, Q], F16, tag="outp", name=f"ot_{b}_{dt}")
            for i, h, k in ihk:
                rhs = (
                    qth[h][:, k - KO, :]
                    if k >= KO
                    else ctxTs[k][:, h * QH : (h + 1) * QH]
                )
                nc.tensor.matmul(
                    ps[h][:],
                    fwt[:, dt, k, :],
                    rhs,
                    start=(i == 0),
                    stop=(i == KE - 1),
                )
                if i == KE - 1:
                    qsl = slice(h * QH, (h + 1) * QH)
                    nc.scalar.activation(
                        ot[:, qsl],
                        ps[h][:],
                        mybir.ActivationFunctionType.Tanh,
                        bias=fbt[:, dt : dt + 1],
                    )
                    # stores ride the scalar DGE queue: they never block loads
                    nc.scalar.dma_start(outT_d[b, dt, :, qsl], ot[:, qsl])

    # batch b+1's loads are emitted right after batch b's m1 so their
    # descriptor ops sit ahead of b's store waits in both DGE rings
    emit_loads(0)
    emit_m1(0)
    for b in range(BL):
        if b + 1 < BL:
            emit_loads(b + 1)
        emit_m2(b)
        emit_m3(b)
        if b + 1 < BL:
            emit_m1(b + 1)


def build_bass():
    nc = bacc.Bacc("TRN2", target_bir_lowering=False, debug=False)
    qT_d = nc.dram_tensor("qT", [BL, 2, P, KO, QH], F16, kind="ExternalInput").ap()
    vT_d = nc.dram_tensor("vT", [BL, ST // 2, P, 2, KO, P], F16, kind="ExternalInput").ap()
    vN8_d = nc.dram_tensor("vN8", [BL, KO, P, NT8, P], F8, kind="ExternalInput").ap()
    vNb_d = nc.dram_tensor("vNb", [BL, KO, P, STB, P], BF16, kind="ExternalInput").ap()
    fw_d = nc.dram_tensor("fw", [KO, P, KE, P], F16, kind="ExternalInput").ap()
    fb_d = nc.dram_tensor("fb", [P, KO], F32, kind="ExternalInput").ap()
    outT_d = nc.dram_tensor("outT", [BL, KO, P, Q], F16, kind="ExternalOutput").ap()

    with tile.TileContext(nc) as tc:
        with ExitStack() as ctx:
            _build_kernel(ctx, tc, qT_d, vT_d, vN8_d, vNb_d, fw_d, fb_d, outT_d)
    nc.compile()
    return nc


def get_compiled():
    global _COMPILED
    if _COMPILED is None:
        _COMPILED = build_bass()
    return _COMPILED


def prep_inputs(queries, values, fc_w, fc_b):
    """Host-side reshape/transposes into the per-core tiled DMA layouts."""
    import ml_dtypes

    queries = np.ascontiguousarray(queries, dtype=np.float32)
    values = np.ascontiguousarray(values, dtype=np.float32)
    fc_w = np.ascontiguousarray(fc_w, dtype=np.float32)
    fc_b = np.ascontiguousarray(fc_b, dtype=np.float32)

    # qT[b,h,p,k,qh] = Q[b,h*QH+qh,128k+p]  (h-major: 8KB-contiguous SBUF rows)
    qT = np.ascontiguousarray(
        queries.transpose(0, 2, 1)
        .reshape(B, KO, P, 2, QH)
        .transpose(0, 3, 2, 1, 4),
        dtype=np.float16,
    )
    # vT[b,tp,p,u,k,s] = V[b,128*(2tp+u)+s,128k+p]  (t-pairs: 4KB DMA rows)
    vT = np.ascontiguousarray(
        values.transpose(0, 2, 1)
        .reshape(B, KO, P, ST // 2, 2, P)
        .transpose(0, 3, 2, 4, 1, 5),
        dtype=np.float16,
    )
    # vN[b,j,p,t,d] = V[b,128t+p,128j+d]; t<NT8 as fp8-e4m3, rest bf16
    vN = values.reshape(B, ST, P, KO, P).transpose(0, 3, 2, 1, 4)
    vN8 = np.ascontiguousarray(
        np.clip(vN[:, :, :, :NT8, :], -240.0, 240.0)
    ).astype(ml_dtypes.float8_e4m3)
    vNb = np.ascontiguousarray(vN[:, :, :, NT8:, :]).astype(ml_dtypes.bfloat16)
    # fw[dt,p,k,o] = fc_w[128dt+o, 128k+p]
    fw = np.ascontiguousarray(
        fc_w.T.reshape(KE, P, KO, P).transpose(2, 1, 0, 3), dtype=np.float16
    )
    # fb[p,dt] = fc_b[128dt+p]
    fb = np.ascontiguousarray(fc_b.reshape(KO, P).T)

    in_maps = []
    for c in range(NCORES):
        sl = slice(BL * c, BL * (c + 1))
        in_maps.append(
            {
                "qT": np.ascontiguousarray(qT[sl]),
                "vT": np.ascontiguousarray(vT[sl]),
                "vN8": np.ascontiguousarray(vN8[sl]),
                "vNb": np.ascontiguousarray(vNb[sl]),
                "fw": fw,
                "fb": fb,
            }
        )
    return in_maps


def unshard_output(results):
    """results: list of per-core dicts with 'outT' [BL, KO, P, Q] -> [B, Q, D]."""
    outT = np.concatenate(
        [np.asarray(res["outT"]).astype(np.float32) for res in results], axis=0
    )  # [B, KO, P, Q]
    return np.ascontiguousarray(outT.reshape(B, D, Q).transpose(0, 2, 1))


def run(in_maps, retries=3, **kwargs):
    nc = get_compiled()
    last_err = None
    for attempt in range(retries):
        try:
            return run_bass_kernel_spmd(nc, in_maps, list(range(NCORES)), **kwargs)
        except Exception as e:  # transient NRT/axon device errors clear on retry
            last_err = e
            time.sleep(5)
    raise last_err


def _kernel_subprocess(queries, values, fc_w, fc_b):
    """Run the kernel in a fresh process.

    A transient NRT "device unrecoverable" wedge survives in-process retries
    (the axon client keeps the broken state) but always clears on process
    restart, so this is the reliable fallback path."""
    import os
    import subprocess
    import tempfile

    kpath = os.path.abspath(__file__)
    with tempfile.TemporaryDirectory() as td:
        np.save(os.path.join(td, "queries.npy"), queries)
        np.save(os.path.join(td, "values.npy"), values)
        np.save(os.path.join(td, "fc_w.npy"), fc_w)
        np.save(os.path.join(td, "fc_b.npy"), fc_b)
        child = (
            "import importlib.util, numpy as np, sys, os\n"
            f"td = {td!r}\n"
            f"spec = importlib.util.spec_from_file_location('gradkernel', {kpath!r})\n"
            "m = importlib.util.module_from_spec(spec)\n"
            "spec.loader.exec_module(m)\n"
            "args = {n: np.load(os.path.join(td, n + '.npy')) for n in ('queries', 'values', 'fc_w', 'fc_b')}\n"
            "in_maps = m.prep_inputs(**args)\n"
            "res = m.run(in_maps, retries=2)\n"
            "np.save(os.path.join(td, 'out.npy'), m.unshard_output(res.results))\n"
        )
        last = None
        for _ in range(3):
            try:
                subprocess.run(
                    [sys.executable, "-c", child], check=True, timeout=1800
                )
                return np.load(os.path.join(td, "out.npy"))
            except Exception as e:
                last = e
                time.sleep(10)
        raise last


def kernel(queries, values, fc_w, fc_b):
    in_maps = prep_inputs(queries, values, fc_w, fc_b)
    try:
        res = run(in_maps, retries=2)
        return unshard_output(res.results)
    except Exception:
        return _kernel_subprocess(queries, values, fc_w, fc_b)


# revision 17
# speedup vs baseline: 1.0094x; 1.0094x over previous
"""Trainium2 Bass kernel for DotProductAttention + concat-FC (B=16,Q=1024,S=2048,D=1024).

Strategy
--------
Data-parallel over batch: 16 batches / 8 cores = 2 per core, zero collectives.

Per batch, everything is computed in a TRANSPOSED layout so that no on-device
transposes are needed (all operand layouts are produced host-side):

  m1:  scoresT[s,q] = sum_d V[s,d]*Q[q,d]      lhsT = vT tile [d,s], rhs = qT [d,q]
  softmax over s (= partitions), exploiting shift invariance: exp(x - C) with a
      constant C=128 straight off PSUM on ScalarE, per-(s-partition) partial
      sums chained on VectorE, then one gpsimd partition_all_reduce(add),
      then reciprocal.
  align: the NT8 first s-tiles are normalized (exp*recip) and quantized to
      fp8-e4m3 on VectorE; the remaining s-tiles normalized to bf16. A second
      colsum chain over the *quantized* align values + all_reduce gives recip2,
      which renormalizes away the common-mode fp8 quantization error of align.
  m2:  ctxT[d,q]  = sum_s V[s,d]*align[s,q]
      fp8 s-tile PAIRS via DoubleRow matmuls (2 contraction tiles per MM at
      ~0.5 cyc/row) with e4m3 V; bf16 s-tiles as normal matmuls; one PSUM
      accumulation group per (j,h); drain multiplies by recip2 -> ctxT f16.
      m2 runs h-outer (all h0 j-chunks then h1) so VectorE can produce h1's
      align tiles under m2-h0's PE work; h0's align production interleaves
      into m1-h1's t-loop (it only needs recip1-h0).
  m3:  outT[o,q] = tanh(sum_e fc_w[o,e]*combT[e,q] + b[o])
      combT = [ctxT ; qT] picked per contraction chunk, bias+tanh fused in one
      ScalarE activation on the PSUM drain. fp16 throughout (fp8 here would
      inject ~4% straight into the output; budget is 2e-2).

Numerics: NT8=14 of 16 s-tiles in fp8 gives rel_err 1.8834e-2 (numpy-emulated
and HW-verified to 5 digits, bit-stable across runs); NT8=16 would be
2.013e-2 — over the 2e-2 budget. m1 must stay fp16: even 1/8 of the score
contraction in fp8 flips argmaxes of the (very peaked, scores~N(0,32))
softmax. m3 fp8 would inject ~4% straight into the output.

Softmax-stats critical path (the first fp8 attempt lost 22us/batch to it):
  * partition reductions (colsum broadcast) are ONES-MATMULS on the PE
    (ones.T @ x, 213ns) instead of gpsimd partition_all_reduce (4.5us);
  * recip1 is reciprocal_approx_fast (~5x cheaper than reciprocal; ANY
    recip1/colsum error is per-q common-mode on align and cancels exactly
    through recip2's renorm, which is why colsum accumulates in bf16);
  * colsum2 (sum of quantized align) is computed exactly in PSUM by
    ones-matmuls (fp8-DR over al8 pairs + bf16 over alb);
  * DVE queue order at the m1-end boundary: recip1-h1 + gamma cast,
    recip2-h0, drain(j0,h0) (frees its PSUM bank early), then h1's 16
    align mults — m2-h1 is ready ~12us post-m1, inside m2-h0's ~19us
    of PE cover; h0's align mults interleave into m1-h1's per-chain gaps.

Perf notes (fp16 baseline measured ~352us at the fast clock state; fp16 PE
streaming floor for the 1536 N=512 matmuls is ~329us; this version measures
~312-315us: PE busy 297.7us vs ~295.7 floor for this dtype mix, <2us of PE
gaps (all in the DMA-bound head), ~16us head (7.9 engine start + 8.2 for the
critical 1.5MB over 2 DGE rings incl ~3.5us ring spin-up), ~5us sync-epilogue
tail. DoubleRow fp8 pair MMs issue at 216ns — IDENTICAL to a single bf16 MM,
i.e. a TRUE 2x per converted s-tile (the doc's 1.44x is pessimistic here).
NOTE the device has a slow clock state (everything uniformly x1.2) that
persists per-session, plus occasional ~8% interference stretches; compare
runs via the matmul duration histogram: 379ns median = fast state):
  * 16-bit operands for m1/m3 (fp16 q/v/fc_w/ctx, bf16 exp).
  * m1 runs h-outer/t-inner with all 16 V-tiles resident (loaded as t-pairs:
    4KB DMA descriptors), so only qt_h0 (1MB) + one V-pair gates the first
    chain; qt_h1 streams in under the h0 sweep.
  * NWARM dummy N=256 matmuls on memset tiles bridge engine-start (~7.4us) to
    first-data (~13us) so the PE HAM clock-gate (1.2 GHz cold -> 2.4 GHz warm
    after ~3.4us of continuous busy; any ~3us idle re-throttles) is fully
    lifted when real work starts.
  * The DGE rings admit ~3-4 transfers concurrently and FAIR-SHARE bandwidth,
    so every later load is gated behind qt_h0a's arrival via a tiny GpSimd
    write into its tile (WAW dep the scheduler must honor).
  * Loads split across the sync + scalar DGE rings; stores ride the scalar
    ring behind their tanh so they never block load issue.
  * fc_w stays resident in SBUF (4MB fp16) across both batches.
  * m3 contracts the qT half (k=8..15) before the ctxT half so it can start
    before m2's last drains; the final (b1,dt7) group runs its h-chains
    sequentially so the kernel tail is one tanh + one 256KB store.
"""

import sys
import time

if "/opt/trn_rl_repo" not in sys.path:
    sys.path.insert(0, "/opt/trn_rl_repo")

from contextlib import ExitStack

import numpy as np

import concourse.bass as bass  # noqa: F401  (import registers engine classes)
import concourse.mybir as mybir
import concourse.tile as tile
from concourse import bacc
from concourse.bass_utils import run_bass_kernel_spmd

P = 128
B, Q, S, D = 16, 1024, 2048, 1024
NCORES = 8
BL = B // NCORES  # 2 batches per core
QH = Q // 2       # q processed in halves of 512
ST = S // P       # 16 s-tiles
KO = D // P       # 8 contraction chunks over d
KE = 2 * D // P   # 16 contraction chunks over e=2D

NT8 = 14          # s-tiles of m2 computed in fp8-e4m3 (DoubleRow pairs)
NDR = NT8 // 2    # DoubleRow pair count
STB = ST - NT8    # bf16 s-tiles

F32 = mybir.dt.float32
F16 = mybir.dt.float16
BF16 = mybir.dt.bfloat16
F8 = mybir.dt.float8e4
DR = mybir.MatmulPerfMode.DoubleRow

# Constant softmax shift: scores ~ N(0, sqrt(D)=32) so row maxes sit in
# [~70, ~190]; exp(x-128) stays comfortably inside fp32/bf16 range both ways.
SOFTMAX_SHIFT = 128.0

NWARM = 34  # dummy matmuls (N=256) spanning the head DMA window: the HAM
# clock gate needs ~3.4us of continuous PE busy to lift (1.2 -> 2.4 GHz), and
# any >~2.5us idle afterwards re-throttles, so the dummies must bridge all the
# way from engine-start (~7.8us) to first-data (~14.4us)

_COMPILED = None


def _build_kernel(ctx: ExitStack, tc: "tile.TileContext", qT_d, vT_d, vN8_d, vNb_d, fw_d, fb_d, outT_d):
    nc = tc.nc
    consts = ctx.enter_context(tc.tile_pool(name="consts", bufs=1))
    qt_pool = ctx.enter_context(tc.tile_pool(name="qt", bufs=4))
    vt_pool = ctx.enter_context(tc.tile_pool(name="vt", bufs=ST // 2))
    # exps as per-t tiles: at h1-t the window of live tiles is h0[t..15] +
    # h1[0..t] = 17 (h0-t's last reader is its align mult at h1-t), so 18
    # buffers let h1's writes reuse h0's buffers as the sweep progresses.
    pexp = ctx.enter_context(tc.tile_pool(name="pexp", bufs=ST + 2))
    al8_pool = ctx.enter_context(tc.tile_pool(name="al8", bufs=2))
    alb_pool = ctx.enter_context(tc.tile_pool(name="alb", bufs=2))
    stats = ctx.enter_context(tc.tile_pool(name="stats", bufs=2))
    ctx_pool = ctx.enter_context(tc.tile_pool(name="ctxT", bufs=KO))
    vc8_pool = ctx.enter_context(tc.tile_pool(name="vc8", bufs=KO))
    vcb_pool = ctx.enter_context(tc.tile_pool(name="vcb", bufs=KO))
    fw_pool = ctx.enter_context(tc.tile_pool(name="fw", bufs=1))
    outp = ctx.enter_context(tc.tile_pool(name="outp", bufs=2))
    psum = ctx.enter_context(tc.tile_pool(name="psum", bufs=8, space="PSUM"))

    # ---- PE warm-up: dummy matmuls with no DMA dependency ----
    wl = consts.tile([P, P], F16)
    wr = consts.tile([P, QH // 2], F16)
    nc.vector.memset(wl[:], 0.0)
    nc.vector.memset(wr[:], 0.0)
    wp = psum.tile([P, QH], F32, tag="psum", name="warm")[:, : QH // 2]
    for _ in range(NWARM):
        nc.tensor.matmul(wp[:], wl[:], wr[:], start=True, stop=True)

    shift = consts.tile([P, 1], F32)
    nc.vector.memset(shift[:], -float(SOFTMAX_SHIFT))
    fbt = consts.tile([P, KO], F32)
    fwt = fw_pool.tile([P, KO, KE, P], F16)
    # all-ones stationaries: partition reduction as a single PE matmul
    # (ones.T @ x broadcasts sum_p x[p,q] to every output partition) --
    # replaces the 4.5us gpsimd partition_all_reduce on the critical path
    ones_bf = consts.tile([P, P], BF16)
    nc.vector.memset(ones_bf[:], 1.0)
    ones8 = consts.tile([P, 2, P], F8)
    nc.vector.memset(ones8[:], 1.0)

    half = KO // 2
    batches = []

    def emit_loads(b):
        """Batch b's wave-1 loads + (b0) the WAW gates for everything else.

        sync ring:   qt_h0 halves; scalar ring: first two vt pairs + fc_b.
        The rest of the loads are emitted inside emit_m1's t-loop and, for
        b0, WAW-gated on qt_h0a so they can't fair-share against wave-1.
        """
        st = {}
        st["qth"] = [
            qt_pool.tile([P, KO, QH], F16, tag="qt", name=f"qt_{b}_{h}")
            for h in range(2)
        ]
        st["vtp"] = [
            vt_pool.tile([P, 2, KO, P], F16, tag="vt", name=f"vtp_{b}_{tp}")
            for tp in range(ST // 2)
        ]
        st["vcs8"] = [
            vc8_pool.tile([P, NT8, P], F8, tag="vc8", name=f"vc8_{b}_{j}")
            for j in range(KO)
        ]
        st["vcsb"] = [
            vcb_pool.tile([P, STB, P], BF16, tag="vcb", name=f"vcb_{b}_{j}")
            for j in range(KO)
        ]
        # wave-1: ONLY what the first h0 chains need. The DGE rings admit
        # ~3-4 transfers concurrently and fair-share bandwidth between them,
        # so anything issued here delays the critical first bytes 1:1.
        nc.sync.dma_start(st["qth"][0][:, :half, :], qT_d[b, 0, :, :half, :])
        nc.scalar.dma_start(st["vtp"][0][:], vT_d[b, 0])
        nc.sync.dma_start(st["qth"][0][:, half:, :], qT_d[b, 0, :, half:, :])
        nc.scalar.dma_start(st["vtp"][1][:], vT_d[b, 1])
        if b == 0:
            nc.scalar.dma_start(fbt[:], fb_d[:, :])
            # gate every later load behind qt_h0a's arrival: a tiny GpSimd
            # write into each pending tile gives its DMA a WAW dependency, so
            # the DGE rings can't admit them into wave-1, where they would
            # fair-share bandwidth against the critical first megabyte
            gsrc = st["qth"][0][:, 0, :1]
            for tp in range(2, ST // 2):
                nc.gpsimd.tensor_copy(st["vtp"][tp][:, 0, 0, :1], gsrc)
            nc.gpsimd.tensor_copy(st["qth"][1][:, 0, :1], gsrc)
            for j in range(KO):
                nc.gpsimd.tensor_copy(st["vcs8"][j][:, 0, :1], gsrc)
                nc.gpsimd.tensor_copy(st["vcsb"][j][:, 0, :1], gsrc)
            for dt in range(KO):
                nc.gpsimd.tensor_copy(fwt[:, dt, 0, :1], gsrc)
        batches.append(st)
        return st

    def emit_m1(b):
        st = batches[b]
        qth, vtp = st["qth"], st["vtp"]
        exps = st["exps"] = [
            [
                pexp.tile([P, QH], BF16, tag="pexp", name=f"sT_{b}_{h}_{t}")
                for t in range(ST)
            ]
            for h in range(2)
        ]
        al8 = st["al8"] = [
            al8_pool.tile([P, NT8, QH], F8, tag="al8", name=f"al8_{b}_{h}")
            for h in range(2)
        ]
        alb = st["alb"] = [
            alb_pool.tile([P, STB, QH], BF16, tag="alb", name=f"alb_{b}_{h}")
            for h in range(2)
        ]
        # colsum chain accumulates in bf16 (2x DVE; its rounding is a per-q
        # common-mode scale on align that recip2's renorm cancels exactly)
        colsums = st["colsums"] = [
            stats.tile([P, QH], BF16, tag="colsum", name=f"colsum_{b}_{h}")
            for h in range(2)
        ]
        recips1 = st["recips1"] = [
            stats.tile([P, QH], F32, tag="recip1", name=f"recip1_{b}_{h}")
            for h in range(2)
        ]
        gammas = st["gammas"] = [
            stats.tile([P, QH], BF16, tag="gamma", name=f"gamma_{b}_{h}")
            for h in range(2)
        ]
        recips2 = st["recips"] = [
            stats.tile([P, QH], F32, tag="recip2", name=f"recip2_{b}_{h}")
            for h in range(2)
        ]

        def emit_align(h, t):
            # normalized align in the dtype m2 consumes (bf16 x bf16 mult)
            dst = al8[h][:, t, :] if t < NT8 else alb[h][:, t - NT8, :]
            nc.vector.tensor_tensor(
                dst, exps[h][t][:], gammas[h][:], mybir.AluOpType.mult
            )

        st["emit_align"] = emit_align

        for h in range(2):
            for t in range(ST):
                vt = vtp[t // 2][:, t % 2]
                ps = psum.tile([P, QH], F32, tag="psum", name=f"ps_sc_{b}_{h}_{t}")
                for k in range(KO):
                    nc.tensor.matmul(
                        ps[:],
                        vt[:, k, :],
                        qth[h][:, k, :],
                        start=(k == 0),
                        stop=(k == KO - 1),
                    )
                # softmax is shift-invariant: exp(x - C) with a constant C
                nc.scalar.activation(
                    exps[h][t][:],
                    ps[:],
                    mybir.ActivationFunctionType.Exp,
                    bias=shift[:],
                )
                if t == 0:
                    nc.vector.tensor_copy(colsums[h][:], exps[h][0][:])
                else:
                    nc.vector.tensor_tensor(
                        colsums[h][:],
                        colsums[h][:],
                        exps[h][t][:],
                        mybir.AluOpType.add,
                    )
                if h == 1:
                    if t == 0:
                        # h0's softmax stats: ones-matmul allreduce slotted
                        # after h1-t0's chain (covers colsum-h0's DVE tail),
                        # then a cheap approx reciprocal -- any recip1 error
                        # is common-mode per q and cancels via recip2
                        psr = psum.tile([P, QH], F32, tag="psum", name=f"psr_{b}_0")
                        nc.tensor.matmul(
                            psr[:], ones_bf[:], colsums[0][:], start=True, stop=True
                        )
                        nc.vector.reciprocal_approx_fast(recips1[0][:], psr[:])
                        nc.vector.tensor_copy(gammas[0][:], recips1[0][:])
                    else:
                        # h0's align tiles only need gamma-h0: interleave their
                        # DVE ops into h1's sweep (fits in the per-chain gap)
                        emit_align(0, t - 1)
                # remaining loads, gated behind this exp's semaphore wait so
                # their descriptors are admitted at consumption pace and never
                # steal DMA bandwidth from earlier, more urgent transfers
                if h == 0:
                    if t < ST // 2 - 2:
                        nc.scalar.dma_start(vtp[t + 2][:], vT_d[b, t + 2])
                    elif t < ST // 2:
                        qsl = (
                            slice(0, half) if t == ST // 2 - 2 else slice(half, KO)
                        )
                        nc.scalar.dma_start(qth[1][:, qsl, :], qT_d[b, 1, :, qsl, :])
                    else:
                        # vN8 rides the (otherwise idle) sync ring so the
                        # scalar ring keeps room for vt/qt/store traffic
                        nc.sync.dma_start(st["vcs8"][t - ST // 2][:], vN8_d[b, t - ST // 2])
                        nc.scalar.dma_start(st["vcsb"][t - ST // 2][:], vNb_d[b, t - ST // 2])
                elif b == 0 and t < KO:
                    nc.scalar.dma_start(fwt[:, t], fw_d[t])
        emit_align(0, ST - 1)

    def emit_m2(b):
        st = batches[b]
        al8, alb, recips = st["al8"], st["alb"], st["recips"]
        colsums, gammas, recips1 = st["colsums"], st["gammas"], st["recips1"]
        emit_align = st["emit_align"]
        ctxTs = st["ctxTs"] = [
            ctx_pool.tile([P, Q], F16, tag="ctxT", name=f"ctxT_{b}_{j}")
            for j in range(KO)
        ]
        scratch = stats.tile([P, QH], F32, tag="scratch", name=f"scr_{b}")

        def colsum2(h):
            # sum of the QUANTIZED align values, exactly, in PSUM via
            # ones-matmuls: recip2 renormalizes ctx by the true align mass
            psc = psum.tile([P, QH], F32, tag="psum", name=f"psc_{b}_{h}")
            for p in range(NDR):
                nc.tensor.matmul(
                    psc[:],
                    ones8[:],
                    al8[h][:, 2 * p : 2 * p + 2, :],
                    start=(p == 0),
                    stop=False,
                    perf_mode=DR,
                )
            for tb in range(STB):
                nc.tensor.matmul(
                    psc[:],
                    ones_bf[:],
                    alb[h][:, tb, :],
                    start=False,
                    stop=(tb == STB - 1),
                )
            nc.vector.reciprocal_approx_accurate(recips[h][:], psc[:], scratch[:])

        def drain(j, h):
            nc.vector.tensor_tensor(
                ctxTs[j][:, h * QH : (h + 1) * QH],
                psums2[j][h][:],
                recips[h][:],
                mybir.AluOpType.mult,
            )

        psums2 = [[None, None] for _ in range(KO)]
        # h-outer: all of h0's j-chunks run while DVE produces h1's aligns
        for h in range(2):
            for j in range(KO):
                vc8, vcb = st["vcs8"][j], st["vcsb"][j]
                ps = psum.tile([P, QH], F32, tag="psum", name=f"ps_ctx_{b}_{j}_{h}")
                psums2[j][h] = ps
                for p in range(NDR):
                    nc.tensor.matmul(
                        ps[:],
                        vc8[:, 2 * p : 2 * p + 2, :],
                        al8[h][:, 2 * p : 2 * p + 2, :],
                        start=(p == 0),
                        stop=False,
                        perf_mode=DR,
                    )
                for tb in range(STB):
                    nc.tensor.matmul(
                        ps[:],
                        vcb[:, tb, :],
                        alb[h][:, tb, :],
                        start=False,
                        stop=(tb == STB - 1),
                    )
                if h == 0 and j == 0:
                    # PE: h1's allreduce + h0's colsum2, under m2-h0's cover.
                    # DVE order matters (in-order queue): recip1-h1 + gamma,
                    # recip2-h0, drain(0,0) [frees its PSUM bank early], THEN
                    # the 16 h1 align mults -- m2-h1 starts ~12us post-m1,
                    # well inside m2-h0's ~25us of PE work.
                    psr = psum.tile([P, QH], F32, tag="psum", name=f"psr_{b}_1")
                    nc.tensor.matmul(
                        psr[:], ones_bf[:], colsums[1][:], start=True, stop=True
                    )
                    nc.vector.reciprocal_approx_fast(recips1[1][:], psr[:])
                    nc.vector.tensor_copy(gammas[1][:], recips1[1][:])
                    colsum2(0)
                    drain(0, 0)
                    for t in range(ST):
                        emit_align(1, t)
                elif h == 0 and j == 5:
                    colsum2(1)
                    drain(j, h)
                else:
                    drain(j, h)

    def emit_m3(b):
        st = batches[b]
        qth, ctxTs = st["qth"], st["ctxTs"]
        # contract the qT half first: it has no dependency on m2's drains
        korder = list(range(KO, KE)) + list(range(KO))
        for dt in range(KO):
            ps = [
                psum.tile([P, QH], F32, tag="psum", name=f"ps_out_{b}_{dt}_{h}")
                for h in range(2)
            ]
            # the final group (dt=7 of the last batch) runs its h-chains
            # sequentially so the kernel tail is one tanh + one 256KB store
            last = b == BL - 1 and dt == KO - 1
            ihk = (
                [(i, h, k) for h in range(2) for i, k in enumerate(korder)]
                if last
                else [(i, h, k) for i, k in enumerate(korder) for h in range(2)]
            )
            ot = outp.tile([P, Q], F16, tag="outp", name=f"ot_{b}_{dt}")
            for i, h, k in ihk:
                rhs = (
                    qth[h][:, k - KO, :]
                    if k >= KO
                    else ctxTs[k][:, h * QH : (h + 1) * QH]
                )
                nc.tensor.matmul(
                    ps[h][:],
                    fwt[:, dt, k, :],
                    rhs,
                    start=(i == 0),
                    stop=(i == KE - 1),
                )
                if i == KE - 1:
                    qsl = slice(h * QH, (h + 1) * QH)
                    nc.scalar.activation(
                        ot[:, qsl],
                        ps[h][:],
                        mybir.ActivationFunctionType.Tanh,
                        bias=fbt[:, dt : dt + 1],
                    )
                    # stores ride the scalar DGE queue: they never block loads
                    nc.scalar.dma_start(outT_d[b, dt, :, qsl], ot[:, qsl])

    # batch b+1's loads are emitted right after batch b's m1 so their
    # descriptor ops sit ahead of b's store waits in both DGE rings
    emit_loads(0)
    emit_m1(0)
    for b in range(BL):
        if b + 1 < BL:
            emit_loads(b + 1)
        emit_m2(b)
        emit_m3(b)
        if b + 1 < BL:
            emit_m1(b + 1)


def build_bass():
    nc = bacc.Bacc("TRN2", target_bir_lowering=False, debug=False)
    qT_d = nc.dram_tensor("qT", [BL, 2, P, KO, QH], F16, kind="ExternalInput").ap()
    vT_d = nc.dram_tensor("vT", [BL, ST // 2, P, 2, KO, P], F16, kind="ExternalInput").ap()
    vN8_d = nc.dram_tensor("vN8", [BL, KO, P, NT8, P], F8, kind="ExternalInput").ap()
    vNb_d = nc.dram_tensor("vNb", [BL, KO, P, STB, P], BF16, kind="ExternalInput").ap()
    fw_d = nc.dram_tensor("fw", [KO, P, KE, P], F16, kind="ExternalInput").ap()
    fb_d = nc.dram_tensor("fb", [P, KO], F32, kind="ExternalInput").ap()
    outT_d = nc.dram_tensor("outT", [BL, KO, P, Q], F16, kind="ExternalOutput").ap()

    with tile.TileContext(nc) as tc:
        with ExitStack() as ctx:
            _build_kernel(ctx, tc, qT_d, vT_d, vN8_d, vNb_d, fw_d, fb_d, outT_d)
    nc.compile()
    return nc


def get_compiled():
    global _COMPILED
    if _COMPILED is None:
        _COMPILED = build_bass()
    return _COMPILED


def prep_inputs(queries, values, fc_w, fc_b):
    """Host-side reshape/transposes into the per-core tiled DMA layouts."""
    import ml_dtypes

    queries = np.ascontiguousarray(queries, dtype=np.float32)
    values = np.ascontiguousarray(values, dtype=np.float32)
    fc_w = np.ascontiguousarray(fc_w, dtype=np.float32)
    fc_b = np.ascontiguousarray(fc_b, dtype=np.float32)

    # qT[b,h,p,k,qh] = Q[b,h*QH+qh,128k+p]  (h-major: 8KB-contiguous SBUF rows)
    qT = np.ascontiguousarray(
        queries.transpose(0, 2, 1)
        .reshape(B, KO, P, 2, QH)
        .transpose(0, 3, 2, 1, 4),
        dtype=np.float16,
    )
    # vT[b,tp,p,u,k,s] = V[b,128*(2tp+u)+s,128k+p]  (t-pairs: 4KB DMA rows)
    vT = np.ascontiguousarray(
        values.transpose(0, 2, 1)
        .reshape(B, KO, P, ST // 2, 2, P)
        .transpose(0, 3, 2, 4, 1, 5),
        dtype=np.float16,
    )
    # vN[b,j,p,t,d] = V[b,128t+p,128j+d]; t<NT8 as fp8-e4m3, rest bf16
    vN = values.reshape(B, ST, P, KO, P).transpose(0, 3, 2, 1, 4)
    vN8 = np.ascontiguousarray(
        np.clip(vN[:, :, :, :NT8, :], -240.0, 240.0)
    ).astype(ml_dtypes.float8_e4m3)
    vNb = np.ascontiguousarray(vN[:, :, :, NT8:, :]).astype(ml_dtypes.bfloat16)
    # fw[dt,p,k,o] = fc_w[128dt+o, 128k+p]
    fw = np.ascontiguousarray(
        fc_w.T.reshape(KE, P, KO, P).transpose(2, 1, 0, 3), dtype=np.float16
    )
    # fb[p,dt] = fc_b[128dt+p]
    fb = np.ascontiguousarray(fc_b.reshape(KO, P).T)

    in_maps = []
    for c in range(NCORES):
        sl = slice(BL * c, BL * (c + 1))
        in_maps.append(
            {
                "qT": np.ascontiguousarray(qT[sl]),
                "vT": np.ascontiguousarray(vT[sl]),
                "vN8": np.ascontiguousarray(vN8[sl]),
                "vNb": np.ascontiguousarray(vNb[sl]),
                "fw": fw,
                "fb": fb,
            }
        )
    return in_maps


def unshard_output(results):
    """results: list of per-core dicts with 'outT' [BL, KO, P, Q] -> [B, Q, D]."""
    outT = np.concatenate(
        [np.asarray(res["outT"]).astype(np.float32) for res in results], axis=0
    )  # [B, KO, P, Q]
    return np.ascontiguousarray(outT.reshape(B, D, Q).transpose(0, 2, 1))


def run(in_maps, retries=3, **kwargs):
    nc = get_compiled()
    last_err = None
    for attempt in range(retries):
        try:
            return run_bass_kernel_spmd(nc, in_maps, list(range(NCORES)), **kwargs)
        except Exception as e:  # transient NRT/axon device errors clear on retry
            last_err = e
            time.sleep(5)
    raise last_err


def _kernel_subprocess(queries, values, fc_w, fc_b):
    """Run the kernel in a fresh process.

    A transient NRT "device unrecoverable" wedge survives in-process retries
    (the axon client keeps the broken state) but always clears on process
    restart, so this is the reliable fallback path."""
    import os
    import subprocess
    import tempfile

    kpath = os.path.abspath(__file__)
    with tempfile.TemporaryDirectory() as td:
        np.save(os.path.join(td, "queries.npy"), queries)
        np.save(os.path.join(td, "values.npy"), values)
        np.save(os.path.join(td, "fc_w.npy"), fc_w)
        np.save(os.path.join(td, "fc_b.npy"), fc_b)
        child = (
            "import importlib.util, numpy as np, sys, os\n"
            f"td = {td!r}\n"
            f"spec = importlib.util.spec_from_file_location('gradkernel', {kpath!r})\n"
            "m = importlib.util.module_from_spec(spec)\n"
            "spec.loader.exec_module(m)\n"
            "args = {n: np.load(os.path.join(td, n + '.npy')) for n in ('queries', 'values', 'fc_w', 'fc_b')}\n"
            "in_maps = m.prep_inputs(**args)\n"
            "res = m.run(in_maps, retries=2)\n"
            "np.save(os.path.join(td, 'out.npy'), m.unshard_output(res.results))\n"
        )
        last = None
        for _ in range(3):
            try:
                subprocess.run(
                    [sys.executable, "-c", child], check=True, timeout=1800
                )
                return np.load(os.path.join(td, "out.npy"))
            except Exception as e:
                last = e
                time.sleep(10)
        raise last


def kernel(queries, values, fc_w, fc_b):
    in_maps = prep_inputs(queries, values, fc_w, fc_b)
    try:
        res = run(in_maps, retries=2)
        return unshard_output(res.results)
    except Exception:
        return _kernel_subprocess(queries, values, fc_w, fc_b)
